# revision 4
# baseline (speedup 1.0000x reference)
"""Trainium2 kernel for NeuralDictionaryV15 (retrieval_knn, top-1 softmax dictionary).

Reference computation:
    logits = keys @ query            # [N]
    att    = softmax(logits)         # [N]
    mask   = att >= max(att)         # top-1 (ties kept)
    out    = (mask * att) @ values   # [V]

Device work (the only part that must stream big data): logits = keys @ query.
keys is 512MB; values need only the argmax row(s), gathered on host.

Sharding: keys row-sharded across 8 cores (32768 rows each). Each core runs an
identical Bass/Tile kernel: stream its shard through SBUF in 1MB tiles, fused
multiply+row-reduce on the Vector engine (tensor_tensor_reduce), emit 32768
logits. Host assembles the 262144 logits (1MB), does softmax/mask in float32
exactly like the reference, and gathers the masked value rows.
"""

import numpy as np

N = 262144
D = 512
V = 512
NCORES = 8
NSHARD = N // NCORES          # 32768 rows per core
P = 128                       # SBUF partitions
RB = NSHARD // P              # 256 row-blocks per core
B = 4                         # row-blocks per DMA tile (4 * 128 * 512 * 4B = 1MB)

_CACHE = {}


def _patch_tile_drain():
    """Kept for backwards-compat with probes; _split_waits subsumes it."""


def _split_waits(nc):
    """Work around walrus/concourse skew: this walrus build accepts at most
    one semaphore wait per instruction, but Tile emits several. Move extra
    waits onto same-engine nops inserted just before the instruction."""
    import concourse.mybir as mybir
    import bass_rust

    cnt = 0
    for f in nc.m.functions:
        for blk in f.blocks:
            newlist = []
            for ins in blk.instructions:
                si = ins.sync_info
                waits = list(si.on_wait) if si and si.on_wait else []
                if len(waits) > 1:
                    for w in waits[:-1]:
                        nop = bass_rust.InstNoOp(name=f"{ins.name}-wsplit{cnt}")
                        cnt += 1
                        nop.engine = ins.engine
                        nop.sync_info = mybir.SyncInfo(on_wait=[w], on_update=[])
                        newlist.append(nop)
                    ins.sync_info = mybir.SyncInfo(
                        on_wait=[waits[-1]],
                        on_update=list(si.on_update) if si.on_update else [],
                    )
                newlist.append(ins)
            blk.instructions = newlist
    return cnt


def _build_nc():
    import concourse.bass as bass
    import concourse.mybir as mybir
    from concourse.tile import TileContext

    nc = bass.Bass()
    keys = nc.declare_dram_parameter("keys", [NSHARD, D], mybir.dt.float32, isOutput=False)
    query = nc.declare_dram_parameter("query", [1, D], mybir.dt.float32, isOutput=False)
    logits = nc.declare_dram_parameter("logits", [P, RB], mybir.dt.float32, isOutput=True)

    # global shard row index = (i*B + b)*P + p  ->  tile [i] laid out [p, b, d]
    keys_r = keys.rearrange("(n b p) d -> n p b d", p=P, b=B)

    with TileContext(nc) as tc:
        with (
            tc.tile_pool(name="ktiles", bufs=4) as kpool,
            tc.tile_pool(name="scratch", bufs=2) as spool,
            tc.tile_pool(name="singles", bufs=1) as singles,
        ):
            q_tile = singles.tile([P, D], mybir.dt.float32)
            q_bcast = bass.AP(
                tensor=query[:].tensor,
                offset=query[:].offset,
                ap=[[0, P], [1, D]],
            )
            nc.gpsimd.dma_start(out=q_tile[:], in_=q_bcast)

            logits_sb = singles.tile([P, RB], mybir.dt.float32)

            for i in range(RB // B):
                kt = kpool.tile([P, B, D], mybir.dt.float32)
                nc.sync.dma_start(out=kt[:], in_=keys_r[i])
                sc = spool.tile([P, B, D], mybir.dt.float32)
                for b in range(B):
                    j = i * B + b
                    # out = (keys * 1.0) * query ; accum_out = row-sum(out)
                    nc.vector.scalar_tensor_tensor(
                        out=sc[:, b, :],
                        in0=kt[:, b, :],
                        scalar=1.0,
                        in1=q_tile[:],
                        op0=mybir.AluOpType.mult,
                        op1=mybir.AluOpType.mult,
                        accum_out=logits_sb[:, j : j + 1],
                    )
            nc.sync.dma_start(out=logits[:], in_=logits_sb[:])
    _split_waits(nc)
    return nc


def _get_nc():
    if "nc" not in _CACHE:
        _CACHE["nc"] = _build_nc()
    return _CACHE["nc"]


def _run_device(keys, query, trace=False):
    """Run the per-core logits kernel on 8 cores; return full [N] f32 logits."""
    from concourse.bass_utils import run_bass_kernel_spmd

    nc = _get_nc()
    q = np.ascontiguousarray(query.reshape(1, D).astype(np.float32, copy=False))
    in_maps = [
        {
            "keys": np.ascontiguousarray(keys[c * NSHARD : (c + 1) * NSHARD]),
            "query": q,
        }
        for c in range(NCORES)
    ]
    out = run_bass_kernel_spmd(nc, in_maps, core_ids=list(range(NCORES)), trace=trace)
    # logits dram tile [p, j] holds shard row j*128 + p
    logits = np.concatenate(
        [r["logits"].T.reshape(-1) for r in out.results]
    )
    return logits, out


def _finish(logits, values):
    """Replicate the reference softmax/mask/matvec in float32 on host."""
    m = logits.max()
    e = np.exp(logits - m, dtype=np.float32)
    z = e.sum(dtype=np.float32)
    att = e / z
    amax = att.max()
    idx = np.nonzero(att >= amax)[0]
    out = (att[idx][:, None] * values[idx].astype(np.float32)).sum(axis=0)
    return out.astype(np.float32)


def kernel(query, keys, values):
    query = np.asarray(query, dtype=np.float32)
    keys = np.asarray(keys, dtype=np.float32)
    values = np.asarray(values)
    logits, _ = _run_device(keys, query, trace=False)
    return _finish(logits, values)


# revision 5
# speedup vs baseline: 1.0654x; 1.0654x over previous
"""Trainium2 kernel for NeuralDictionaryV15 (retrieval_knn, top-1 softmax dictionary).

Reference computation:
    logits = keys @ query            # [N]
    att    = softmax(logits)         # [N]
    mask   = att >= max(att)         # top-1 (ties kept)
    out    = (mask * att) @ values   # [V]

Device work (the only part that must stream big data): logits = keys @ query.
keys is 512MB; values need only the argmax row(s), gathered on host.

Sharding: keys row-sharded across 8 cores (32768 rows each). Each core runs an
identical Bass/Tile kernel: stream its shard through SBUF in multi-MB tiles,
fused multiply+row-reduce on the Vector engine (scalar_tensor_tensor with
accum_out), emit 32768 logits. Host assembles the 262144 logits (1MB), does
softmax/mask in float32 exactly like the reference, and gathers the masked
value rows.
"""

import numpy as np

N = 262144
D = 512
V = 512
NCORES = 8
NSHARD = N // NCORES          # 32768 rows per core
P = 128                       # SBUF partitions
RB = NSHARD // P              # 256 row-blocks per core

# tunables
B = 8                         # row-blocks per DMA tile (8 -> 2MB per DMA)
KBUFS = 6                     # key tile buffers
DUMMY_OUT = True              # write STT product to a [P,1] stride-0 dummy
ALT_DMA = False               # alternate sync/scalar HWDGE rings

_CACHE = {}


def _split_waits(nc):
    """Work around walrus/concourse skew: this walrus build accepts at most
    one semaphore wait per instruction, but Tile emits several. Move extra
    waits onto same-engine nops inserted just before the instruction."""
    import concourse.mybir as mybir
    import bass_rust

    cnt = 0
    for f in nc.m.functions:
        for blk in f.blocks:
            newlist = []
            for ins in blk.instructions:
                si = ins.sync_info
                waits = list(si.on_wait) if si and si.on_wait else []
                if len(waits) > 1:
                    for w in waits[:-1]:
                        nop = bass_rust.InstNoOp(name=f"{ins.name}-wsplit{cnt}")
                        cnt += 1
                        nop.engine = ins.engine
                        nop.sync_info = mybir.SyncInfo(on_wait=[w], on_update=[])
                        newlist.append(nop)
                    ins.sync_info = mybir.SyncInfo(
                        on_wait=[waits[-1]],
                        on_update=list(si.on_update) if si.on_update else [],
                    )
                newlist.append(ins)
            blk.instructions = newlist
    return cnt


def _build_nc(b=None, kbufs=None, dummy_out=None, alt_dma=None):
    import concourse.bass as bass
    import concourse.mybir as mybir
    from concourse.tile import TileContext

    b = B if b is None else b
    kbufs = KBUFS if kbufs is None else kbufs
    dummy_out = DUMMY_OUT if dummy_out is None else dummy_out
    alt_dma = ALT_DMA if alt_dma is None else alt_dma

    nc = bass.Bass()
    keys = nc.declare_dram_parameter("keys", [NSHARD, D], mybir.dt.float32, isOutput=False)
    query = nc.declare_dram_parameter("query", [1, D], mybir.dt.float32, isOutput=False)
    logits = nc.declare_dram_parameter("logits", [P, RB], mybir.dt.float32, isOutput=True)

    # global shard row index = (i*b + bb)*P + p  ->  tile [i] laid out [p, bb, d]
    keys_r = keys.rearrange("(n b p) d -> n p b d", p=P, b=b)

    with TileContext(nc) as tc:
        with (
            tc.tile_pool(name="ktiles", bufs=kbufs) as kpool,
            tc.tile_pool(name="scratch", bufs=2) as spool,
            tc.tile_pool(name="singles", bufs=1) as singles,
        ):
            q_tile = singles.tile([P, D], mybir.dt.float32)
            q_bcast = bass.AP(
                tensor=query[:].tensor,
                offset=query[:].offset,
                ap=[[0, P], [1, D]],
            )
            nc.gpsimd.dma_start(out=q_tile[:], in_=q_bcast)

            logits_sb = singles.tile([P, RB], mybir.dt.float32)

            for i in range(RB // b):
                kt = kpool.tile([P, b, D], mybir.dt.float32)
                eng = nc.scalar if (alt_dma and i % 2) else nc.sync
                eng.dma_start(out=kt[:], in_=keys_r[i])
                if dummy_out:
                    sc = spool.tile([P, 1], mybir.dt.float32)
                else:
                    sc = spool.tile([P, b, D], mybir.dt.float32)
                for bb in range(b):
                    j = i * b + bb
                    out_ap = sc.broadcast_to((P, D)) if dummy_out else sc[:, bb, :]
                    # out = (keys * 1.0) * query ; accum_out = row-sum(out)
                    nc.vector.scalar_tensor_tensor(
                        out=out_ap,
                        in0=kt[:, bb, :],
                        scalar=1.0,
                        in1=q_tile[:],
                        op0=mybir.AluOpType.mult,
                        op1=mybir.AluOpType.mult,
                        accum_out=logits_sb[:, j : j + 1],
                    )
            nc.sync.dma_start(out=logits[:], in_=logits_sb[:])
    _split_waits(nc)
    return nc


def _get_nc():
    if "nc" not in _CACHE:
        _CACHE["nc"] = _build_nc()
    return _CACHE["nc"]


def _run_device(keys, query, trace=False, nc=None):
    """Run the per-core logits kernel on 8 cores; return full [N] f32 logits."""
    from concourse.bass_utils import run_bass_kernel_spmd

    if nc is None:
        nc = _get_nc()
    q = np.ascontiguousarray(query.reshape(1, D).astype(np.float32, copy=False))
    in_maps = [
        {
            "keys": np.ascontiguousarray(keys[c * NSHARD : (c + 1) * NSHARD]),
            "query": q,
        }
        for c in range(NCORES)
    ]
    out = run_bass_kernel_spmd(nc, in_maps, core_ids=list(range(NCORES)), trace=trace)
    # logits dram tile [p, j] holds shard row j*128 + p
    logits = np.concatenate(
        [r["logits"].T.reshape(-1) for r in out.results]
    )
    return logits, out


def _finish(logits, values):
    """Replicate the reference softmax/mask/matvec in float32 on host."""
    m = logits.max()
    e = np.exp(logits - m, dtype=np.float32)
    z = e.sum(dtype=np.float32)
    att = e / z
    amax = att.max()
    idx = np.nonzero(att >= amax)[0]
    out = (att[idx][:, None] * values[idx].astype(np.float32)).sum(axis=0)
    return out.astype(np.float32)


def kernel(query, keys, values):
    query = np.asarray(query, dtype=np.float32)
    keys = np.asarray(keys, dtype=np.float32)
    values = np.asarray(values)
    logits, _ = _run_device(keys, query, trace=False)
    return _finish(logits, values)


# revision 7
# speedup vs baseline: 3.2426x; 3.0434x over previous
"""Trainium2 kernel for NeuralDictionaryV15 (retrieval_knn, top-1 softmax dictionary).

Reference computation:
    logits = keys @ query            # [N]
    att    = softmax(logits)         # [N]
    mask   = att >= max(att)         # top-1 (ties kept)
    out    = (mask * att) @ values   # [V]

Device work (the only part that must stream big data): logits = keys @ query.
keys is 512MB; values need only the argmax row(s), gathered on host.

Sharding: keys row-sharded across 8 cores (32768 rows each). Each core runs an
identical Bass/Tile kernel: stream its shard through SBUF in multi-MB tiles,
fused multiply+row-reduce on the Vector engine (scalar_tensor_tensor with
accum_out), emit 32768 logits. Host assembles the 262144 logits (1MB), does
softmax/mask in float32 exactly like the reference, and gathers the masked
value rows.
"""

import numpy as np

N = 262144
D = 512
V = 512
NCORES = 8
NSHARD = N // NCORES          # 32768 rows per core
P = 128                       # SBUF partitions
RB = NSHARD // P              # 256 row-blocks per core

# tunables
B = 8                         # row-blocks per DMA tile (8 -> 2MB per DMA)
KBUFS = 6                     # key tile buffers
DUMMY_OUT = True              # write STT product to a [P,1] stride-0 dummy
ALT_DMA = False               # alternate sync/scalar HWDGE rings

_CACHE = {}


def _split_waits(nc):
    """Work around walrus/concourse skew: this walrus build accepts at most
    one semaphore wait per instruction, but Tile emits several. Move extra
    waits onto same-engine nops inserted just before the instruction."""
    import concourse.mybir as mybir
    import bass_rust

    cnt = 0
    for f in nc.m.functions:
        for blk in f.blocks:
            newlist = []
            for ins in blk.instructions:
                si = ins.sync_info
                waits = list(si.on_wait) if si and si.on_wait else []
                if len(waits) > 1:
                    for w in waits[:-1]:
                        nop = bass_rust.InstNoOp(name=f"{ins.name}-wsplit{cnt}")
                        cnt += 1
                        nop.engine = ins.engine
                        nop.sync_info = mybir.SyncInfo(on_wait=[w], on_update=[])
                        newlist.append(nop)
                    ins.sync_info = mybir.SyncInfo(
                        on_wait=[waits[-1]],
                        on_update=list(si.on_update) if si.on_update else [],
                    )
                newlist.append(ins)
            blk.instructions = newlist
    return cnt


def _build_nc(b=None, kbufs=None, dummy_out=None, alt_dma=None, kdt="f32"):
    import concourse.bass as bass
    import concourse.mybir as mybir
    from concourse.tile import TileContext

    b = B if b is None else b
    kbufs = KBUFS if kbufs is None else kbufs
    dummy_out = DUMMY_OUT if dummy_out is None else dummy_out
    alt_dma = ALT_DMA if alt_dma is None else alt_dma
    kdtype = mybir.dt.float32 if kdt == "f32" else mybir.dt.bfloat16

    nc = bass.Bass()
    keys = nc.declare_dram_parameter("keys", [NSHARD, D], kdtype, isOutput=False)
    query = nc.declare_dram_parameter("query", [1, D], kdtype, isOutput=False)
    logits = nc.declare_dram_parameter("logits", [P, RB], mybir.dt.float32, isOutput=True)

    # global shard row index = (i*b + bb)*P + p  ->  tile [i] laid out [p, bb, d]
    keys_r = keys.rearrange("(n b p) d -> n p b d", p=P, b=b)

    with TileContext(nc) as tc:
        with (
            tc.tile_pool(name="ktiles", bufs=kbufs) as kpool,
            tc.tile_pool(name="scratch", bufs=2) as spool,
            tc.tile_pool(name="singles", bufs=1) as singles,
        ):
            q_tile = singles.tile([P, D], kdtype)
            q_bcast = bass.AP(
                tensor=query[:].tensor,
                offset=query[:].offset,
                ap=[[0, P], [1, D]],
            )
            nc.gpsimd.dma_start(out=q_tile[:], in_=q_bcast)

            logits_sb = singles.tile([P, RB], mybir.dt.float32)

            for i in range(RB // b):
                kt = kpool.tile([P, b, D], kdtype)
                eng = nc.scalar if (alt_dma and i % 2) else nc.sync
                eng.dma_start(out=kt[:], in_=keys_r[i])
                if dummy_out:
                    sc = spool.tile([P, 1], kdtype)
                else:
                    sc = spool.tile([P, b, D], kdtype)
                for bb in range(b):
                    j = i * b + bb
                    out_ap = sc.broadcast_to((P, D)) if dummy_out else sc[:, bb, :]
                    # out = (keys * 1.0) * query ; accum_out = row-sum(out)
                    nc.vector.scalar_tensor_tensor(
                        out=out_ap,
                        in0=kt[:, bb, :],
                        scalar=1.0,
                        in1=q_tile[:],
                        op0=mybir.AluOpType.mult,
                        op1=mybir.AluOpType.mult,
                        accum_out=logits_sb[:, j : j + 1],
                    )
            nc.sync.dma_start(out=logits[:], in_=logits_sb[:])
    _split_waits(nc)
    return nc


def _get_nc():
    if "nc" not in _CACHE:
        _CACHE["nc"] = _build_nc()
    return _CACHE["nc"]


def _run_device(keys, query, trace=False, nc=None):
    """Run the per-core logits kernel on 8 cores; return full [N] f32 logits."""
    from concourse.bass_utils import run_bass_kernel_spmd

    if nc is None:
        nc = _get_nc()
    q = np.ascontiguousarray(query.reshape(1, D).astype(np.float32, copy=False))
    in_maps = [
        {
            "keys": np.ascontiguousarray(keys[c * NSHARD : (c + 1) * NSHARD]),
            "query": q,
        }
        for c in range(NCORES)
    ]
    out = run_bass_kernel_spmd(nc, in_maps, core_ids=list(range(NCORES)), trace=trace)
    # logits dram tile [p, j] holds shard row j*128 + p
    logits = np.concatenate(
        [r["logits"].T.reshape(-1) for r in out.results]
    )
    return logits, out


def _finish(logits, values):
    """Replicate the reference softmax/mask/matvec in float32 on host."""
    m = logits.max()
    e = np.exp(logits - m, dtype=np.float32)
    z = e.sum(dtype=np.float32)
    att = e / z
    amax = att.max()
    idx = np.nonzero(att >= amax)[0]
    out = (att[idx][:, None] * values[idx].astype(np.float32)).sum(axis=0)
    return out.astype(np.float32)


def kernel(query, keys, values):
    query = np.asarray(query, dtype=np.float32)
    keys = np.asarray(keys, dtype=np.float32)
    values = np.asarray(values)
    logits, _ = _run_device(keys, query, trace=False)
    return _finish(logits, values)
